# revision 21
# baseline (speedup 1.0000x reference)
"""Trainium2 Bass kernel for nn_ModelAttention2Layers (B=8, S=2048, D=512, K=256).

Only batch 0 matters (the reference returns final[0, -1, :]), so the 2048-query
sequence of batch 0 is sharded across the 8 cores (256 queries each).

All cross-core data movement uses relative-addressed remote_dma_broadcast
(SBUF -> SBUF) instead of collective_compute (15us fixed overhead + 40GB/s
each in the perf model). Each allgather is a tc.Switch on the core id: core j
issues one 8-destination broadcast (self included) whose out slot is j, so
slot j always holds core j's shard (keys stay in linear order):
  - block 1's k1 shards (k1 = Wk1^T x is sharded, not recomputed 8x; this
    also removes the 4MB full-xT load),
  - block 2's k2 and v2 shards,
  - hidden[-1] (core 7's last row; 16B/partition),
  - block 3's flash-style [o|l] partials, summed on every core.
Receive-side ordering: a Pool nop "gate" gets a wait on the remote semaphore
injected AFTER tile scheduling (the scheduling sim cannot satisfy
remotely-incremented semaphores), and every reader of a gathered tile gets an
explicit dependency edge on the gate.

Activation-table discipline: only {Exp, Ln, Square, Copy} are used (one
act-func table -> no 1.3us table reloads); 1/sqrt(s) = exp(-0.5*ln(s)).
Matmuls run in float32r / bf16 (full PE rate); k-projection biases are dropped
(softmax-invariant; they are zero in setup_inputs anyway).
"""
import sys

sys.path.insert(0, "/opt/trn_rl_repo")

import numpy as np

S, D, K, P, C = 2048, 512, 256, 128, 8
SH = S // C          # 256 queries/keys per core
ND, NK, NS, NSH = D // P, K // P, S // P, SH // P   # 4, 2, 16, 2
TRN2_NC_BASE = (0, 1, 2, 3, 6, 7, 4, 5)
RDESTS = [(0, TRN2_NC_BASE[s]) for s in range(C)]   # relative, self included
RSEM_TARGET = C * (16 // C)                          # 8 senders x 2

# packed-constants layout (one [P, PACKW] f32 DMA): see _pack() below
COL_BQ1, COL_BQ2 = 0, NK
COL_ONESCOL = 2 * NK
COL_NEGHALF = 2 * NK + 1
COL_CID = 2 * NK + 2
COL_IDENT = 2 * NK + 3
COL_BV2 = COL_IDENT + P          # row 0 only
COL_ONESROW = COL_BV2 + D        # row 0 only
PACKW = COL_ONESROW + P

_cache = {}


def _steer_act_tables():
    """Make the act-table insertion pass resolve Ln to
    natural_log_exp_and_others (which also holds Exp/Square/Copy) instead of
    first-match natural_log: hide Ln from every other table. Table order and
    indices are unchanged, so the emitted act_func_set_id still addresses the
    real act_info.json entry (which genuinely contains Ln). Returns a restore
    thunk."""
    import concourse.bacc as bacc_mod
    from concourse import mybir

    orig = bacc_mod.get_activation_tables

    def steered(arch):
        tabs = dict(orig(arch))
        ln = mybir.ActivationFunctionType.Ln
        exp = mybir.ActivationFunctionType.Exp
        for name, funcs in tabs.items():
            if ln in funcs and exp not in funcs:
                tabs[name] = funcs - {ln}
            elif exp in funcs and ln not in funcs:
                tabs[name] = funcs - {exp}
        return tabs

    bacc_mod.get_activation_tables = steered
    return lambda: setattr(bacc_mod, "get_activation_tables", orig)


def _build():
    import concourse.bass as bass
    import concourse.tile as tile
    from concourse import mybir, bacc
    from bass_rust import add_dep_helper, RuntimeValue

    F32 = mybir.dt.float32
    F32R = mybir.dt.float32r
    BF16 = mybir.dt.bfloat16
    I32 = mybir.dt.int32
    AF = mybir.ActivationFunctionType
    ts = bass.ts

    nc = bacc.Bacc()

    ins = {}
    for name, shape, dt in [
        ("x0", [S, D], F32), ("xTq", [D, SH], F32),
        ("Wk1", [D, K], F32), ("Wq1", [D, K], F32), ("Wk2", [D, K], F32),
        ("Wq2", [D, K], F32), ("Wv2", [D, D], F32),
        ("pack", [P, PACKW], F32),
    ]:
        ins[name] = nc.dram_tensor(name, shape, dt, kind="ExternalInput")
    out_ext = nc.dram_tensor("out", [D], F32, kind="ExternalOutput")

    gates = []  # (nop instruction, semaphore, target) -> wait injected post-schedule

    with tile.TileContext(nc) as tc:
        with tc.tile_pool(name="const", bufs=1) as cw, \
             tc.tile_pool(name="big", bufs=1) as big, \
             tc.tile_pool(name="work", bufs=1) as wk, \
             tc.tile_pool(name="send", bufs=1) as snd, \
             tc.tile_pool(name="gath", bufs=1) as gth, \
             tc.tile_pool(name="pp", bufs=2) as pp, \
             tc.tile_pool(name="small", bufs=2) as sm, \
             tc.tile_pool(name="scp", bufs=2, space="PSUM") as scp, \
             tc.tile_pool(name="mmp", bufs=2, space="PSUM") as mmp, \
             tc.tile_pool(name="tpp", bufs=2, space="PSUM") as tpp:

            rsem_k1 = nc.alloc_semaphore("rsem_k1")
            rsem_k2 = nc.alloc_semaphore("rsem_k2")
            rsem_v2 = nc.alloc_semaphore("rsem_v2")
            rsem_hl = nc.alloc_semaphore("rsem_hl")
            rsem_ol = nc.alloc_semaphore("rsem_ol")
            lsem = nc.alloc_semaphore("lsem")

            # ---- input loads ----
            # f32r tiles are loaded with a bitcast (same bits); SP carries
            # xTq + x0, Act carries the weights + the packed constants.
            # Pool stays free for remote-DMA desc generation.
            xTq_r = cw.tile([P, ND, SH], F32R)
            nc.sync.dma_start(xTq_r[:],
                              ins["xTq"][:].bitcast(F32R).rearrange("(k p) j -> p k j", p=P))
            x0_r = big.tile([P, NS, D], F32R, tag="XV")
            x0_dmas = []
            for cb in range(4):
                x0_dmas.append(nc.sync.dma_start(
                    x0_r[:, 4 * cb:4 * cb + 4, :],
                    ins["x0"][:].bitcast(F32R).rearrange("(n p) d -> p n d", p=P)[:, 4 * cb:4 * cb + 4, :]))
            pk = cw.tile([P, PACKW], F32)
            nc.scalar.dma_start(pk[:], ins["pack"][:])
            W_r = {}
            w_dmas = {}
            for w, ncol in [("Wk1", K), ("Wq1", K), ("Wk2", K), ("Wq2", K),
                            ("Wv2", D)]:
                W_r[w] = cw.tile([P, ND, ncol], F32R, name=f"W_{w}", tag=f"W_{w}")
                w_dmas[w] = nc.scalar.dma_start(
                    W_r[w][:], ins[w][:].bitcast(F32R).rearrange("(k p) n -> p k n", p=P))
            for w in ("Wk2", "Wq2", "Wv2"):
                add_dep_helper(w_dmas[w].ins, x0_dmas[-1].ins, sync=True,
                               reason="x0 before late weights")
            bq1_sb = pk[:, COL_BQ1:COL_BQ1 + NK]
            bq2_sb = pk[:, COL_BQ2:COL_BQ2 + NK]
            onescol_f = pk[:, COL_ONESCOL:COL_ONESCOL + 1]
            neghalf = pk[:, COL_NEGHALF:COL_NEGHALF + 1]
            ident_r = pk[:, COL_IDENT:COL_IDENT + P].bitcast(F32R)
            bv2_r = pk[0:1, COL_BV2:COL_BV2 + D].bitcast(F32R)
            ones_r = pk[0:1, COL_ONESROW:COL_ONESROW + P].bitcast(F32R)

            cid_reg = nc.gpsimd.alloc_register("cid")
            nc.gpsimd.reg_load(cid_reg, pk[0:1, COL_CID:COL_CID + 1].bitcast(I32))
            cid_val = RuntimeValue(cid_reg, min_val=0, max_val=C - 1)

            triggers = []

            def bcast_send(full_tile, src_tile, rsem, name):
                """Switch on core id; core j broadcasts src into slot j of
                full_tile on all 8 cores (self included)."""
                for j in tc.Switch(cid_val, C, hint=f"ag_{name}"):
                    nc.gpsimd.remote_dma_broadcast(
                        full_tile[:, j], src_tile[:],
                        remote_sem=rsem, local_sem=lsem, rdests=RDESTS)
                    triggers.append(nc.gpsimd.trigger_dma(count=None))

            def make_gate(rsem, name):
                """Pool nop that (post-scheduling) waits for all 8 broadcasts.
                Ordered after every trigger emitted so far, so a blocked gate
                never delays a send."""
                gate = nc.gpsimd.nop(nofuse=True, hint=f"gate_{name}")
                for t in triggers:
                    add_dep_helper(gate.ins, t.ins, sync=False,
                                   reason="sends before gate")
                gates.append((gate, rsem, RSEM_TARGET))
                return gate

            # ---- block 1 projections (sharded k1) ----
            k1s = snd.tile([P, NK, SH], F32R, tag="snd_k1")
            for m in range(NK):
                pm = mmp.tile([P, SH], F32, tag="mm")
                for k in range(ND):
                    nc.tensor.matmul(pm[:], W_r["Wk1"][:, k, ts(m, P)], xTq_r[:, k, :],
                                     start=(k == 0), stop=(k == ND - 1))
                nc.scalar.activation(k1s[:, m, :], pm[:], AF.Copy)
            k1_full = gth.tile([P, C, NK, SH], F32R, tag="g_k1", name="dbg_k1full")
            bcast_send(k1_full, k1s, rsem_k1, "k1")
            gate_k1 = make_gate(rsem_k1, "k1")

            q1T = wk.tile([P, NK, SH], F32R, tag="qT1", name="dbg_q1T")
            for m in range(NK):
                pm = mmp.tile([P, SH], F32, tag="mm")
                for k in range(ND):
                    nc.tensor.matmul(pm[:], W_r["Wq1"][:, k, ts(m, P)], xTq_r[:, k, :],
                                     start=(k == 0), stop=(k == ND - 1))
                nc.vector.tensor_scalar_add(q1T[:, m, :], pm[:], bq1_sb[:, m:m + 1])

            def attention(qT, kfull, v_blocks, out_dst, pt_dtype,
                          score_gate=None, av_gate=None, pool_reduce=False):
                """out_dst[:, qm, :] = softmax(q.k^T) @ V for this core's
                queries (keys in linear order). The two query blocks are
                software-pipelined through two [P, 1024] psum score slots."""
                def scores(qm, h):
                    sch = scp.tile([P, 4, SH], F32, tag="sc")
                    for jj in range(4):
                        j = 4 * h + jj
                        for dm in range(NK):
                            mm = nc.tensor.matmul(
                                sch[:, jj, :], qT[:, dm, ts(qm, P)],
                                kfull[:, j, dm, :],
                                start=(dm == 0), stop=(dm == NK - 1))
                            if score_gate is not None:
                                add_dep_helper(mm.ins, score_gate.ins,
                                               sync=True, reason="gathered keys")
                    return sch

                def creduce(sch, mx4, h):
                    nc.vector.reduce_max(mx4[:, h:h + 1],
                                         sch[:].rearrange("p a b -> p (a b)"),
                                         axis=mybir.AxisListType.X)

                def neg_max(mx4):
                    mx = sm.tile([P, 1], F32, tag="mx")
                    nc.vector.reduce_max(mx[:], mx4[:], axis=mybir.AxisListType.X)
                    nm = sm.tile([P, 1], F32, tag="nm")
                    nc.vector.tensor_scalar_mul(nm[:], mx[:], -1.0)
                    return nm

                def exp_half(sch, Pt, lsum, nm, h):
                    flat = sch[:].rearrange("p a b -> p (a b)")
                    nc.scalar.activation(Pt[:, h * 1024:(h + 1) * 1024], flat[:],
                                         AF.Exp, bias=nm[:],
                                         accum_out=lsum[:, h:h + 1])

                def transp(Pt, PT, glo, ghi):
                    for g in range(glo, ghi):
                        tp = tpp.tile([P, 4, P], F32R, tag="tp")
                        for u in range(4):
                            nc.tensor.transpose(tp[:, u, :], Pt[:, ts(4 * g + u, P)],
                                                ident_r)
                        dstap = PT[:, 4 * g:4 * g + 4, :].rearrange("p a b -> p (a b)")
                        srcap = tp[:].rearrange("p a b -> p (a b)")
                        if g % 2 == 0:
                            nc.vector.tensor_copy(dstap, srcap)
                        else:
                            nc.scalar.activation(dstap, srcap, AF.Copy)

                def av_out(PT, lsum, qm):
                    l = sm.tile([P, 1], F32, tag="l")
                    nc.vector.reduce_sum(l[:], lsum[:], axis=mybir.AxisListType.X)
                    rl = sm.tile([P, 1], F32, tag="rl")
                    nc.vector.reciprocal(rl[:], l[:])
                    av = mmp.tile([P, D], F32, tag="mm")
                    for n in range(NS):
                        mm = nc.tensor.matmul(av[:], PT[:, n, :], v_blocks[n],
                                              start=(n == 0), stop=(n == NS - 1))
                        if av_gate is not None:
                            add_dep_helper(mm.ins, av_gate.ins, sync=True,
                                           reason="gathered values")
                    nc.scalar.activation(out_dst[:, qm, :], av[:], AF.Copy,
                                         scale=rl[:])

                mx4_0 = sm.tile([P, 2], F32, tag="mx4")
                mx4_1 = sm.tile([P, 2], F32, tag="mx4")
                s0a = scores(0, 0); creduce(s0a, mx4_0, 0)
                s0b = scores(0, 1); creduce(s0b, mx4_0, 1)
                nm0 = neg_max(mx4_0)
                Pt0 = pp.tile([P, S], F32R, tag="P")
                lsum0 = sm.tile([P, 2], F32, tag="lsum")
                exp_half(s0a, Pt0, lsum0, nm0, 0)
                s1a = scores(1, 0); creduce(s1a, mx4_1, 0)
                exp_half(s0b, Pt0, lsum0, nm0, 1)
                PT0 = pp.tile([P, NS, P], pt_dtype, tag="PT")
                transp(Pt0, PT0, 0, 2)
                s1b = scores(1, 1); creduce(s1b, mx4_1, 1)
                transp(Pt0, PT0, 2, 4)
                av_out(PT0, lsum0, 0)
                nm1 = neg_max(mx4_1)
                Pt1 = pp.tile([P, S], F32R, tag="P")
                lsum1 = sm.tile([P, 2], F32, tag="lsum")
                exp_half(s1a, Pt1, lsum1, nm1, 0)
                exp_half(s1b, Pt1, lsum1, nm1, 1)
                PT1 = pp.tile([P, NS, P], pt_dtype, tag="PT")
                transp(Pt1, PT1, 0, 4)
                av_out(PT1, lsum1, 1)

            out1 = wk.tile([P, NSH, D], F32R, tag="H", name="dbg_out1")
            attention(q1T, k1_full, [x0_r[:, n, :] for n in range(NS)], out1, F32R,
                      score_gate=gate_k1, pool_reduce=True)

            def transpose_rows(src, hl_out=None):
                """src [P, NSH, D] -> dst [P, ND, SH]; optionally extract the
                last row (query SH-1) into hl_out [P, ND]."""
                dst = wk.tile([P, ND, SH], F32R, tag="HT")
                for qm in range(NSH):
                    tp = tpp.tile([P, 4, P], F32R, tag="tp")
                    for dm in range(ND):
                        nc.tensor.transpose(tp[:, dm, :], src[:, qm, ts(dm, P)],
                                            ident_r)
                    if qm == 0:
                        nc.scalar.activation(dst[:, :, ts(qm, P)], tp[:], AF.Copy)
                    else:
                        nc.vector.tensor_copy(dst[:, :, ts(qm, P)], tp[:])
                    if hl_out is not None and qm == NSH - 1:
                        nc.vector.tensor_copy(
                            hl_out[:],
                            tp[:, :, P - 1:P].rearrange("p a b -> p (a b)"))
                return dst

            out1T = transpose_rows(out1)

            # ---- block 2 shard projections + gathers ----
            k2T = snd.tile([P, NK, SH], BF16, tag="snd_k2")
            for m in range(NK):
                pm = mmp.tile([P, SH], F32, tag="mm")
                for k in range(ND):
                    nc.tensor.matmul(pm[:], W_r["Wk2"][:, k, ts(m, P)], out1T[:, k, :],
                                     start=(k == 0), stop=(k == ND - 1))
                nc.scalar.activation(k2T[:, m, :], pm[:], AF.Copy)
            k2_full = gth.tile([P, C, NK, SH], BF16, tag="g_k2", name="dbg_k2full")
            bcast_send(k2_full, k2T, rsem_k2, "k2")
            gate_k2 = make_gate(rsem_k2, "k2")

            q2T = wk.tile([P, NK, SH], BF16, tag="qT2")
            for m in range(NK):
                pm = mmp.tile([P, SH], F32, tag="mm")
                for k in range(ND):
                    nc.tensor.matmul(pm[:], W_r["Wq2"][:, k, ts(m, P)], out1T[:, k, :],
                                     start=(k == 0), stop=(k == ND - 1))
                nc.vector.tensor_scalar_add(q2T[:, m, :], pm[:], bq2_sb[:, m:m + 1])

            def rsqrt_act(dstap, srcap):
                """1/sqrt(s) via exp(-0.5*ln(s)) — stays in one act table."""
                t = sm.tile([P, 1], F32, tag="lnt")
                nc.scalar.activation(t[:], srcap, AF.Ln)
                nc.scalar.activation(dstap, t[:], AF.Exp, scale=neghalf)

            def vproj(hT, out_dtype, pool, tag, normalize):
                """v = h @ Wv2 + bv2 for this core's 256 rows; if normalize,
                rows are L2-normalized, else the raw rows and the 1/|row|
                factors (rn [P, NSH]) are returned separately."""
                v_sb = pool.tile([P, NSH, D], out_dtype, tag=tag)
                rn_t = None if normalize else sm.tile([P, NSH], F32, tag="rn3")
                for r in range(NSH):
                    pm = mmp.tile([P, D], F32, tag="mm")
                    for k in range(ND):
                        nc.tensor.matmul(pm[:], hT[:, k, ts(r, P)], W_r["Wv2"][:, k, :],
                                         start=(k == 0), stop=False)
                    nc.tensor.matmul(pm[:], ones_r, bv2_r, start=False, stop=True)
                    scr = sm.tile([P, D], F32, tag="scr")
                    ssum = sm.tile([P, 1], F32, tag="ssum")
                    nc.scalar.activation(scr[:], pm[:], AF.Square, accum_out=ssum[:])
                    if normalize:
                        rn = sm.tile([P, 1], F32, tag="rn")
                        rsqrt_act(rn[:], ssum[:])
                        nc.vector.tensor_scalar_mul(v_sb[:, r, :], pm[:], rn[:])
                    else:
                        rsqrt_act(rn_t[:, r:r + 1], ssum[:])
                        nc.vector.tensor_copy(v_sb[:, r, :], pm[:])
                return v_sb, rn_t

            v2, _ = vproj(out1T, BF16, snd, "snd_v2", normalize=True)
            v2_full = gth.tile([P, C, NSH, D], BF16, tag="g_v2", name="dbg_v2full")
            bcast_send(v2_full, v2, rsem_v2, "v2")
            gate_v2 = make_gate(rsem_v2, "v2")

            # ---- block 2 attention ----
            v2_blocks = [v2_full[:, n // NSH, n % NSH, :] for n in range(NS)]
            hidden = wk.tile([P, NSH, D], F32R, tag="H", name="dbg_hidden")
            attention(q2T, k2_full, v2_blocks, hidden, BF16,
                      score_gate=gate_k2, av_gate=gate_v2)

            hl_c = snd.tile([P, ND], F32R, tag="snd_hl")
            hT = transpose_rows(hidden, hl_out=hl_c)

            hlg = gth.tile([P, C, ND], F32R, tag="g_hl", name="dbg_hlg")
            bcast_send(hlg, hl_c, rsem_hl, "hl")
            gate_hl = make_gate(rsem_hl, "hl")

            # ---- block 3 (flash-style partials over this core's 256 keys).
            # k3/v3/rn3 only need local data and overlap the hl exchange; the
            # 1/|v| factors are folded into p3 so nothing heavy sits on the
            # post-hl critical path.
            k3T = wk.tile([P, NK, SH], F32R, tag="k3")
            for m in range(NK):
                pm = mmp.tile([P, SH], F32, tag="mm")
                for k in range(ND):
                    nc.tensor.matmul(pm[:], W_r["Wk2"][:, k, ts(m, P)], hT[:, k, :],
                                     start=(k == 0), stop=(k == ND - 1))
                nc.vector.tensor_copy(k3T[:, m, :], pm[:])
            v3, rn3 = vproj(hT, F32R, wk, "v3", normalize=False)

            # q3 = Wq2^T @ hidden[-1] + bq2 ; hidden[-1] is core 7's slot
            q3 = sm.tile([P, NK], F32R, tag="q3")
            for fm in range(NK):
                pm = mmp.tile([P, 1], F32, tag="mm")
                for dm in range(ND):
                    mm = nc.tensor.matmul(pm[:], W_r["Wq2"][:, dm, ts(fm, P)],
                                          hlg[:, C - 1, dm:dm + 1],
                                          start=(dm == 0), stop=(dm == ND - 1))
                    add_dep_helper(mm.ins, gate_hl.ins, sync=True, reason="hl gather")
                nc.vector.tensor_scalar_add(q3[:, fm:fm + 1], pm[:], bq2_sb[:, fm:fm + 1])

            # s3 (scores for my 256 keys; |s3| small so exp needs no max shift)
            s3p = tpp.tile([P, NSH], F32, tag="tp")
            for n in range(NSH):
                for fm in range(NK):
                    nc.tensor.matmul(s3p[:, n:n + 1], k3T[:, fm, ts(n, P)],
                                     q3[:, fm:fm + 1],
                                     start=(fm == 0), stop=(fm == NK - 1))
            p3e = sm.tile([P, NSH], F32, tag="p3e")
            nc.scalar.activation(p3e[:], s3p[:], AF.Exp)
            p3 = sm.tile([P, NSH], F32R, tag="p3")
            nc.vector.tensor_tensor(p3[:], p3e[:], rn3[:], mybir.AluOpType.mult)

            # partial numerator oT [128,4] (d on partitions) + replicated l
            ol_ps = mmp.tile([P, ND + 1], F32, tag="mm")
            for dm in range(ND):
                for n in range(NSH):
                    nc.tensor.matmul(ol_ps[:, dm:dm + 1], v3[:, n, ts(dm, P)],
                                     p3[:, n:n + 1],
                                     start=(n == 0), stop=(n == NSH - 1))
            l3p = tpp.tile([1, 1], F32, tag="tp")
            for n in range(NSH):
                nc.tensor.matmul(l3p[:], p3e[:, n:n + 1], onescol_f,
                                 start=(n == 0), stop=(n == NSH - 1))
            l3f = sm.tile([1, 1], F32R, tag="l3f")
            nc.vector.tensor_copy(l3f[:], l3p[:])
            nc.tensor.matmul(ol_ps[:, ND:ND + 1], ones_r, l3f[:],
                             start=True, stop=True)
            ol = snd.tile([P, ND + 1], F32, tag="snd_ol")
            nc.vector.tensor_copy(ol[:], ol_ps[:])

            olg = gth.tile([P, C, ND + 1], F32, tag="g_ol", name="dbg_olg")
            bcast_send(olg, ol, rsem_ol, "ol")
            gate_ol = make_gate(rsem_ol, "ol")

            tot = wk.tile([P, ND + 1], F32, tag="tot")
            rs = nc.vector.reduce_sum(tot[:], olg[:].rearrange("p c e -> p e c"),
                                      axis=mybir.AxisListType.X)
            add_dep_helper(rs.ins, gate_ol.ins, sync=True, reason="ol gather")
            rl3 = sm.tile([P, 1], F32, tag="rl3")
            nc.vector.reciprocal(rl3[:], tot[:, ND:ND + 1])
            fin = wk.tile([P, ND], F32, tag="fin")
            nc.vector.tensor_scalar_mul(fin[:], tot[:, 0:ND], rl3[:])
            nc.sync.dma_start(out_ext[:].rearrange("(k p) -> p k", p=P), fin[:])

    for gate, sem, target in gates:
        gate.wait_op(sem, target, "sem-ge")
    restore = _steer_act_tables()
    try:
        nc.finalize()
    finally:
        restore()
    return nc


def _pack(c, f):
    pk = np.zeros((P, PACKW), np.float32)
    pk[:, COL_BQ1:COL_BQ1 + NK] = f("bq1").reshape(NK, P).T
    pk[:, COL_BQ2:COL_BQ2 + NK] = f("bq2").reshape(NK, P).T
    pk[:, COL_ONESCOL] = 1.0
    pk[:, COL_NEGHALF] = -0.5
    pk[:, COL_CID] = np.array([c], np.int32).view(np.float32)[0]
    pk[:, COL_IDENT:COL_IDENT + P] = np.eye(P, dtype=np.float32)
    pk[0, COL_BV2:COL_BV2 + D] = f("bv2")
    pk[0, COL_ONESROW:COL_ONESROW + P] = 1.0
    return pk


def kernel(**inputs):
    from concourse.bass_utils import run_bass_kernel_spmd

    f = lambda k: np.ascontiguousarray(np.asarray(inputs[k], dtype=np.float32))
    x0 = f("x")[0]                       # [S, D]; batches 1..7 are dead
    xT = np.ascontiguousarray(x0.T)      # [D, S]
    base = {
        "x0": x0,
        "Wk1": f("Wk1"), "Wq1": f("Wq1"), "Wk2": f("Wk2"), "Wq2": f("Wq2"),
        "Wv2": f("Wv2"),
    }
    in_maps = [
        {**base,
         "xTq": np.ascontiguousarray(xT[:, c * SH:(c + 1) * SH]),
         "pack": _pack(c, f)}
        for c in range(C)
    ]

    if "nc" not in _cache:
        _cache["nc"] = _build()
    res = run_bass_kernel_spmd(_cache["nc"], in_maps, list(range(C)))
    return res.results[0]["out"].astype(np.float32)


if __name__ == "__main__":
    d = np.load("/root/problem/inputs.npz")
    out = kernel(**{k: d[k] for k in d.files})
    ref = np.load("/root/problem/ref_out.npy")
    rel = np.abs(out - ref).max() / np.abs(ref).max()
    print("Relative error:", rel)


# revision 22
# speedup vs baseline: 1.0386x; 1.0386x over previous
"""Trainium2 Bass kernel for nn_ModelAttention2Layers (B=8, S=2048, D=512, K=256).

Only batch 0 matters (the reference returns final[0, -1, :]), so the 2048-query
sequence of batch 0 is sharded across the 8 cores (256 queries each).

All cross-core data movement uses relative-addressed remote_dma_broadcast
(SBUF -> SBUF) instead of collective_compute (15us fixed overhead + 40GB/s
each in the perf model). Each allgather is a tc.Switch on the core id: core j
issues one 8-destination broadcast (self included) whose out slot is j, so
slot j always holds core j's shard (keys stay in linear order):
  - block 1's k1 shards (k1 = Wk1^T x is sharded, not recomputed 8x; this
    also removes the 4MB full-xT load),
  - block 2's k2 and v2 shards,
  - hidden[-1] (core 7's last row; 16B/partition),
  - block 3's flash-style [o|l] partials, summed on every core.
Receive-side ordering: a Pool nop "gate" gets a wait on the remote semaphore
injected AFTER tile scheduling (the scheduling sim cannot satisfy
remotely-incremented semaphores), and every reader of a gathered tile gets an
explicit dependency edge on the gate.

Activation-table discipline: only {Exp, Ln, Square, Copy} are used (one
act-func table -> no 1.3us table reloads); 1/sqrt(s) = exp(-0.5*ln(s)).
Matmuls run in float32r / bf16 (full PE rate); k-projection biases are dropped
(softmax-invariant; they are zero in setup_inputs anyway).
"""
import sys

sys.path.insert(0, "/opt/trn_rl_repo")

import numpy as np

S, D, K, P, C = 2048, 512, 256, 128, 8
SH = S // C          # 256 queries/keys per core
ND, NK, NS, NSH = D // P, K // P, S // P, SH // P   # 4, 2, 16, 2
TRN2_NC_BASE = (0, 1, 2, 3, 6, 7, 4, 5)
RDESTS = [(0, TRN2_NC_BASE[s]) for s in range(C)]   # relative, self included
RSEM_TARGET = C * (16 // C)                          # 8 senders x 2

# packed-constants layout (one [P, PACKW] f32 DMA): see _pack() below
COL_BQ1, COL_BQ2 = 0, NK
COL_ONESCOL = 2 * NK
COL_NEGHALF = 2 * NK + 1
COL_CID = 2 * NK + 2
COL_IDENT = 2 * NK + 3
COL_BV2 = COL_IDENT + P          # row 0 only
COL_ONESROW = COL_BV2 + D        # row 0 only
PACKW = COL_ONESROW + P

_cache = {}


def _steer_act_tables():
    """Make the act-table insertion pass resolve Ln to
    natural_log_exp_and_others (which also holds Exp/Square/Copy) instead of
    first-match natural_log: hide Ln from every other table. Table order and
    indices are unchanged, so the emitted act_func_set_id still addresses the
    real act_info.json entry (which genuinely contains Ln). Returns a restore
    thunk."""
    import concourse.bacc as bacc_mod
    from concourse import mybir

    orig = bacc_mod.get_activation_tables

    def steered(arch):
        tabs = dict(orig(arch))
        ln = mybir.ActivationFunctionType.Ln
        exp = mybir.ActivationFunctionType.Exp
        for name, funcs in tabs.items():
            if ln in funcs and exp not in funcs:
                tabs[name] = funcs - {ln}
            elif exp in funcs and ln not in funcs:
                tabs[name] = funcs - {exp}
        return tabs

    bacc_mod.get_activation_tables = steered
    return lambda: setattr(bacc_mod, "get_activation_tables", orig)


def _build():
    import concourse.bass as bass
    import concourse.tile as tile
    from concourse import mybir, bacc
    from bass_rust import add_dep_helper, RuntimeValue

    F32 = mybir.dt.float32
    F32R = mybir.dt.float32r
    BF16 = mybir.dt.bfloat16
    I32 = mybir.dt.int32
    AF = mybir.ActivationFunctionType
    ts = bass.ts

    nc = bacc.Bacc()

    ins = {}
    for name, shape, dt in [
        ("x0", [S, D], F32), ("xTq", [D, SH], F32),
        ("Wk1", [D, K], F32), ("Wq1", [D, K], F32), ("Wk2", [D, K], F32),
        ("Wq2", [D, K], F32), ("Wv2", [D, D], F32),
        ("pack", [P, PACKW], F32),
    ]:
        ins[name] = nc.dram_tensor(name, shape, dt, kind="ExternalInput")
    out_ext = nc.dram_tensor("out", [D], F32, kind="ExternalOutput")

    gates = []  # (nop instruction, semaphore, target) -> wait injected post-schedule

    with tile.TileContext(nc) as tc:
        with tc.tile_pool(name="const", bufs=1) as cw, \
             tc.tile_pool(name="big", bufs=1) as big, \
             tc.tile_pool(name="work", bufs=1) as wk, \
             tc.tile_pool(name="send", bufs=1) as snd, \
             tc.tile_pool(name="gath", bufs=1) as gth, \
             tc.tile_pool(name="pp", bufs=2) as pp, \
             tc.tile_pool(name="small", bufs=2) as sm, \
             tc.tile_pool(name="scp", bufs=2, space="PSUM") as scp, \
             tc.tile_pool(name="mmp", bufs=2, space="PSUM") as mmp, \
             tc.tile_pool(name="tpp", bufs=2, space="PSUM") as tpp:

            rsem_k1 = nc.alloc_semaphore("rsem_k1")
            rsem_k2 = nc.alloc_semaphore("rsem_k2")
            rsem_v2 = nc.alloc_semaphore("rsem_v2")
            rsem_hl = nc.alloc_semaphore("rsem_hl")
            rsem_ol = nc.alloc_semaphore("rsem_ol")
            lsem = nc.alloc_semaphore("lsem")

            # ---- input loads ----
            # f32r tiles are loaded with a bitcast (same bits); SP carries
            # xTq + x0, Act carries the weights + the packed constants.
            # Pool stays free for remote-DMA desc generation.
            xTq_r = cw.tile([P, ND, SH], F32R)
            nc.sync.dma_start(xTq_r[:],
                              ins["xTq"][:].bitcast(F32R).rearrange("(k p) j -> p k j", p=P))
            x0_r = big.tile([P, NS, D], F32R, tag="XV")
            x0_dmas = []
            for cb in range(4):
                x0_dmas.append(nc.sync.dma_start(
                    x0_r[:, 4 * cb:4 * cb + 4, :],
                    ins["x0"][:].bitcast(F32R).rearrange("(n p) d -> p n d", p=P)[:, 4 * cb:4 * cb + 4, :]))
            pk = cw.tile([P, PACKW], F32)
            nc.scalar.dma_start(pk[:], ins["pack"][:])
            W_r = {}
            w_dmas = {}
            for w, ncol in [("Wk1", K), ("Wq1", K), ("Wk2", K), ("Wq2", K),
                            ("Wv2", D)]:
                W_r[w] = cw.tile([P, ND, ncol], F32R, name=f"W_{w}", tag=f"W_{w}")
                w_dmas[w] = nc.scalar.dma_start(
                    W_r[w][:], ins[w][:].bitcast(F32R).rearrange("(k p) n -> p k n", p=P))
            for w in ("Wk2", "Wq2", "Wv2"):
                add_dep_helper(w_dmas[w].ins, x0_dmas[-1].ins, sync=True,
                               reason="x0 before late weights")
            bq1_sb = pk[:, COL_BQ1:COL_BQ1 + NK]
            bq2_sb = pk[:, COL_BQ2:COL_BQ2 + NK]
            onescol_f = pk[:, COL_ONESCOL:COL_ONESCOL + 1]
            neghalf = pk[:, COL_NEGHALF:COL_NEGHALF + 1]
            ident_r = pk[:, COL_IDENT:COL_IDENT + P].bitcast(F32R)
            bv2_r = pk[0:1, COL_BV2:COL_BV2 + D].bitcast(F32R)
            ones_r = pk[0:1, COL_ONESROW:COL_ONESROW + P].bitcast(F32R)

            cid_reg = nc.gpsimd.alloc_register("cid")
            nc.gpsimd.reg_load(cid_reg, pk[0:1, COL_CID:COL_CID + 1].bitcast(I32))
            cid_val = RuntimeValue(cid_reg, min_val=0, max_val=C - 1)

            triggers = []

            def bcast_send(full_tile, src_tile, rsem, name):
                """Switch on core id; core j broadcasts src into slot j of
                full_tile on all 8 cores (self included)."""
                for j in tc.Switch(cid_val, C, hint=f"ag_{name}"):
                    nc.gpsimd.remote_dma_broadcast(
                        full_tile[:, j], src_tile[:],
                        remote_sem=rsem, local_sem=lsem, rdests=RDESTS)
                    triggers.append(nc.gpsimd.trigger_dma(count=None))

            def make_gate(rsem, name):
                """Pool nop that (post-scheduling) waits for all 8 broadcasts.
                Ordered after every trigger emitted so far, so a blocked gate
                never delays a send."""
                gate = nc.gpsimd.nop(nofuse=True, hint=f"gate_{name}")
                for t in triggers:
                    add_dep_helper(gate.ins, t.ins, sync=False,
                                   reason="sends before gate")
                gates.append((gate, rsem, RSEM_TARGET))
                return gate

            # ---- block 1 projections (sharded k1) ----
            k1s = snd.tile([P, NK, SH], F32R, tag="snd_k1")
            for m in range(NK):
                pm = mmp.tile([P, SH], F32, tag="mm")
                for k in range(ND):
                    nc.tensor.matmul(pm[:], W_r["Wk1"][:, k, ts(m, P)], xTq_r[:, k, :],
                                     start=(k == 0), stop=(k == ND - 1))
                nc.scalar.activation(k1s[:, m, :], pm[:], AF.Copy)
            k1_full = gth.tile([P, C, NK, SH], F32R, tag="g_k1", name="dbg_k1full")
            bcast_send(k1_full, k1s, rsem_k1, "k1")
            gate_k1 = make_gate(rsem_k1, "k1")

            q1T = wk.tile([P, NK, SH], F32R, tag="qT1", name="dbg_q1T")
            for m in range(NK):
                pm = mmp.tile([P, SH], F32, tag="mm")
                for k in range(ND):
                    nc.tensor.matmul(pm[:], W_r["Wq1"][:, k, ts(m, P)], xTq_r[:, k, :],
                                     start=(k == 0), stop=(k == ND - 1))
                nc.vector.tensor_scalar_add(q1T[:, m, :], pm[:], bq1_sb[:, m:m + 1])

            def attention(qT, kfull, v_blocks, out_dst, pt_dtype,
                          score_gate=None, av_gate=None, pool_reduce=False):
                """out_dst[:, qm, :] = softmax(q.k^T) @ V for this core's
                queries (keys in linear order). The two query blocks are
                software-pipelined through two [P, 1024] psum score slots."""
                def scores(qm, h):
                    sch = scp.tile([P, 4, SH], F32, tag="sc")
                    for jj in range(4):
                        j = 4 * h + jj
                        for dm in range(NK):
                            mm = nc.tensor.matmul(
                                sch[:, jj, :], qT[:, dm, ts(qm, P)],
                                kfull[:, j, dm, :],
                                start=(dm == 0), stop=(dm == NK - 1))
                            if score_gate is not None:
                                add_dep_helper(mm.ins, score_gate.ins,
                                               sync=True, reason="gathered keys")
                    return sch

                def creduce(sch, mx4, h):
                    nc.vector.reduce_max(mx4[:, h:h + 1],
                                         sch[:].rearrange("p a b -> p (a b)"),
                                         axis=mybir.AxisListType.X)

                def neg_max(mx4):
                    mx = sm.tile([P, 1], F32, tag="mx")
                    nc.vector.reduce_max(mx[:], mx4[:], axis=mybir.AxisListType.X)
                    nm = sm.tile([P, 1], F32, tag="nm")
                    nc.vector.tensor_scalar_mul(nm[:], mx[:], -1.0)
                    return nm

                def exp_half(sch, Pt, lsum, nm, h):
                    flat = sch[:].rearrange("p a b -> p (a b)")
                    nc.scalar.activation(Pt[:, h * 1024:(h + 1) * 1024], flat[:],
                                         AF.Exp, bias=nm[:],
                                         accum_out=lsum[:, h:h + 1])

                def transp(Pt, PT, glo, ghi):
                    for g in range(glo, ghi):
                        tp = tpp.tile([P, 4, P], F32R, tag="tp")
                        for u in range(4):
                            nc.tensor.transpose(tp[:, u, :], Pt[:, ts(4 * g + u, P)],
                                                ident_r)
                        nc.vector.tensor_copy(
                            PT[:, 4 * g:4 * g + 4, :].rearrange("p a b -> p (a b)"),
                            tp[:].rearrange("p a b -> p (a b)"))

                def av_out(PT, lsum, qm):
                    l = sm.tile([P, 1], F32, tag="l")
                    nc.vector.reduce_sum(l[:], lsum[:], axis=mybir.AxisListType.X)
                    rl = sm.tile([P, 1], F32, tag="rl")
                    nc.vector.reciprocal(rl[:], l[:])
                    av = mmp.tile([P, D], F32, tag="mm")
                    for n in range(NS):
                        mm = nc.tensor.matmul(av[:], PT[:, n, :], v_blocks[n],
                                              start=(n == 0), stop=(n == NS - 1))
                        if av_gate is not None:
                            add_dep_helper(mm.ins, av_gate.ins, sync=True,
                                           reason="gathered values")
                    nc.scalar.activation(out_dst[:, qm, :], av[:], AF.Copy,
                                         scale=rl[:])

                mx4_0 = sm.tile([P, 2], F32, tag="mx4")
                mx4_1 = sm.tile([P, 2], F32, tag="mx4")
                s0a = scores(0, 0); creduce(s0a, mx4_0, 0)
                s0b = scores(0, 1); creduce(s0b, mx4_0, 1)
                nm0 = neg_max(mx4_0)
                Pt0 = pp.tile([P, S], F32R, tag="P")
                lsum0 = sm.tile([P, 2], F32, tag="lsum")
                exp_half(s0a, Pt0, lsum0, nm0, 0)
                s1a = scores(1, 0); creduce(s1a, mx4_1, 0)
                exp_half(s0b, Pt0, lsum0, nm0, 1)
                PT0 = pp.tile([P, NS, P], pt_dtype, tag="PT")
                transp(Pt0, PT0, 0, 2)
                s1b = scores(1, 1); creduce(s1b, mx4_1, 1)
                transp(Pt0, PT0, 2, 4)
                av_out(PT0, lsum0, 0)
                nm1 = neg_max(mx4_1)
                Pt1 = pp.tile([P, S], F32R, tag="P")
                lsum1 = sm.tile([P, 2], F32, tag="lsum")
                exp_half(s1a, Pt1, lsum1, nm1, 0)
                exp_half(s1b, Pt1, lsum1, nm1, 1)
                PT1 = pp.tile([P, NS, P], pt_dtype, tag="PT")
                transp(Pt1, PT1, 0, 4)
                av_out(PT1, lsum1, 1)

            out1 = wk.tile([P, NSH, D], F32R, tag="H", name="dbg_out1")
            attention(q1T, k1_full, [x0_r[:, n, :] for n in range(NS)], out1, F32R,
                      score_gate=gate_k1, pool_reduce=True)

            def transpose_rows(src, hl_out=None):
                """src [P, NSH, D] -> dst [P, ND, SH]; optionally extract the
                last row (query SH-1) into hl_out [P, ND]."""
                dst = wk.tile([P, ND, SH], F32R, tag="HT")
                for qm in range(NSH):
                    tp = tpp.tile([P, 4, P], F32R, tag="tp")
                    for dm in range(ND):
                        nc.tensor.transpose(tp[:, dm, :], src[:, qm, ts(dm, P)],
                                            ident_r)
                    if qm == 0:
                        nc.scalar.activation(dst[:, :, ts(qm, P)], tp[:], AF.Copy)
                    else:
                        nc.vector.tensor_copy(dst[:, :, ts(qm, P)], tp[:])
                    if hl_out is not None and qm == NSH - 1:
                        nc.vector.tensor_copy(
                            hl_out[:],
                            tp[:, :, P - 1:P].rearrange("p a b -> p (a b)"))
                return dst

            out1T = transpose_rows(out1)

            # ---- block 2 shard projections + gathers ----
            k2T = snd.tile([P, NK, SH], BF16, tag="snd_k2")
            for m in range(NK):
                pm = mmp.tile([P, SH], F32, tag="mm")
                for k in range(ND):
                    nc.tensor.matmul(pm[:], W_r["Wk2"][:, k, ts(m, P)], out1T[:, k, :],
                                     start=(k == 0), stop=(k == ND - 1))
                nc.scalar.activation(k2T[:, m, :], pm[:], AF.Copy)
            k2_full = gth.tile([P, C, NK, SH], BF16, tag="g_k2", name="dbg_k2full")
            bcast_send(k2_full, k2T, rsem_k2, "k2")
            gate_k2 = make_gate(rsem_k2, "k2")

            q2T = wk.tile([P, NK, SH], BF16, tag="qT2")
            for m in range(NK):
                pm = mmp.tile([P, SH], F32, tag="mm")
                for k in range(ND):
                    nc.tensor.matmul(pm[:], W_r["Wq2"][:, k, ts(m, P)], out1T[:, k, :],
                                     start=(k == 0), stop=(k == ND - 1))
                nc.vector.tensor_scalar_add(q2T[:, m, :], pm[:], bq2_sb[:, m:m + 1])

            def rsqrt_act(dstap, srcap):
                """1/sqrt(s) via exp(-0.5*ln(s)) — stays in one act table."""
                t = sm.tile([P, 1], F32, tag="lnt")
                nc.scalar.activation(t[:], srcap, AF.Ln)
                nc.scalar.activation(dstap, t[:], AF.Exp, scale=neghalf)

            def vproj(hT, out_dtype, pool, tag, normalize):
                """v = h @ Wv2 + bv2 for this core's 256 rows; if normalize,
                rows are L2-normalized, else the raw rows and the 1/|row|
                factors (rn [P, NSH]) are returned separately."""
                v_sb = pool.tile([P, NSH, D], out_dtype, tag=tag)
                rn_t = None if normalize else sm.tile([P, NSH], F32, tag="rn3")
                for r in range(NSH):
                    pm = mmp.tile([P, D], F32, tag="mm")
                    for k in range(ND):
                        nc.tensor.matmul(pm[:], hT[:, k, ts(r, P)], W_r["Wv2"][:, k, :],
                                         start=(k == 0), stop=False)
                    nc.tensor.matmul(pm[:], ones_r, bv2_r, start=False, stop=True)
                    scr = sm.tile([P, D], F32, tag="scr")
                    ssum = sm.tile([P, 1], F32, tag="ssum")
                    nc.scalar.activation(scr[:], pm[:], AF.Square, accum_out=ssum[:])
                    if normalize:
                        rn = sm.tile([P, 1], F32, tag="rn")
                        rsqrt_act(rn[:], ssum[:])
                        nc.vector.tensor_scalar_mul(v_sb[:, r, :], pm[:], rn[:])
                    else:
                        rsqrt_act(rn_t[:, r:r + 1], ssum[:])
                        nc.vector.tensor_copy(v_sb[:, r, :], pm[:])
                return v_sb, rn_t

            v2, _ = vproj(out1T, BF16, snd, "snd_v2", normalize=True)
            v2_full = gth.tile([P, C, NSH, D], BF16, tag="g_v2", name="dbg_v2full")
            bcast_send(v2_full, v2, rsem_v2, "v2")
            gate_v2 = make_gate(rsem_v2, "v2")

            # ---- block 2 attention ----
            v2_blocks = [v2_full[:, n // NSH, n % NSH, :] for n in range(NS)]
            hidden = wk.tile([P, NSH, D], F32R, tag="H", name="dbg_hidden")
            attention(q2T, k2_full, v2_blocks, hidden, BF16,
                      score_gate=gate_k2, av_gate=gate_v2)

            hl_c = snd.tile([P, ND], F32R, tag="snd_hl")
            hT = transpose_rows(hidden, hl_out=hl_c)

            hlg = gth.tile([P, C, ND], F32R, tag="g_hl", name="dbg_hlg")
            bcast_send(hlg, hl_c, rsem_hl, "hl")
            gate_hl = make_gate(rsem_hl, "hl")

            # ---- block 3 (flash-style partials over this core's 256 keys).
            # k3/v3/rn3 only need local data and overlap the hl exchange; the
            # 1/|v| factors are folded into p3 so nothing heavy sits on the
            # post-hl critical path.
            k3T = wk.tile([P, NK, SH], F32R, tag="k3")
            for m in range(NK):
                pm = mmp.tile([P, SH], F32, tag="mm")
                for k in range(ND):
                    nc.tensor.matmul(pm[:], W_r["Wk2"][:, k, ts(m, P)], hT[:, k, :],
                                     start=(k == 0), stop=(k == ND - 1))
                nc.vector.tensor_copy(k3T[:, m, :], pm[:])
            v3, rn3 = vproj(hT, F32R, wk, "v3", normalize=False)

            # q3 = Wq2^T @ hidden[-1] + bq2 ; hidden[-1] is core 7's slot
            q3 = sm.tile([P, NK], F32R, tag="q3")
            for fm in range(NK):
                pm = mmp.tile([P, 1], F32, tag="mm")
                for dm in range(ND):
                    mm = nc.tensor.matmul(pm[:], W_r["Wq2"][:, dm, ts(fm, P)],
                                          hlg[:, C - 1, dm:dm + 1],
                                          start=(dm == 0), stop=(dm == ND - 1))
                    add_dep_helper(mm.ins, gate_hl.ins, sync=True, reason="hl gather")
                nc.vector.tensor_scalar_add(q3[:, fm:fm + 1], pm[:], bq2_sb[:, fm:fm + 1])

            # s3 (scores for my 256 keys; |s3| small so exp needs no max shift)
            s3p = tpp.tile([P, NSH], F32, tag="tp")
            for n in range(NSH):
                for fm in range(NK):
                    nc.tensor.matmul(s3p[:, n:n + 1], k3T[:, fm, ts(n, P)],
                                     q3[:, fm:fm + 1],
                                     start=(fm == 0), stop=(fm == NK - 1))
            p3e = sm.tile([P, NSH], F32, tag="p3e")
            nc.scalar.activation(p3e[:], s3p[:], AF.Exp)
            p3 = sm.tile([P, NSH], F32R, tag="p3")
            nc.vector.tensor_tensor(p3[:], p3e[:], rn3[:], mybir.AluOpType.mult)

            # partial numerator oT [128,4] (d on partitions) + replicated l
            ol_ps = mmp.tile([P, ND + 1], F32, tag="mm")
            for dm in range(ND):
                for n in range(NSH):
                    nc.tensor.matmul(ol_ps[:, dm:dm + 1], v3[:, n, ts(dm, P)],
                                     p3[:, n:n + 1],
                                     start=(n == 0), stop=(n == NSH - 1))
            l3p = tpp.tile([1, 1], F32, tag="tp")
            for n in range(NSH):
                nc.tensor.matmul(l3p[:], p3e[:, n:n + 1], onescol_f,
                                 start=(n == 0), stop=(n == NSH - 1))
            l3f = sm.tile([1, 1], F32R, tag="l3f")
            nc.vector.tensor_copy(l3f[:], l3p[:])
            nc.tensor.matmul(ol_ps[:, ND:ND + 1], ones_r, l3f[:],
                             start=True, stop=True)
            ol = snd.tile([P, ND + 1], F32, tag="snd_ol")
            nc.vector.tensor_copy(ol[:], ol_ps[:])

            olg = gth.tile([P, C, ND + 1], F32, tag="g_ol", name="dbg_olg")
            bcast_send(olg, ol, rsem_ol, "ol")
            gate_ol = make_gate(rsem_ol, "ol")

            tot = wk.tile([P, ND + 1], F32, tag="tot")
            rs = nc.vector.reduce_sum(tot[:], olg[:].rearrange("p c e -> p e c"),
                                      axis=mybir.AxisListType.X)
            add_dep_helper(rs.ins, gate_ol.ins, sync=True, reason="ol gather")
            rl3 = sm.tile([P, 1], F32, tag="rl3")
            nc.vector.reciprocal(rl3[:], tot[:, ND:ND + 1])
            fin = wk.tile([P, ND], F32, tag="fin")
            nc.vector.tensor_scalar_mul(fin[:], tot[:, 0:ND], rl3[:])
            nc.sync.dma_start(out_ext[:].rearrange("(k p) -> p k", p=P), fin[:])

    for gate, sem, target in gates:
        gate.wait_op(sem, target, "sem-ge")
    restore = _steer_act_tables()
    try:
        nc.finalize()
    finally:
        restore()
    return nc


def _pack(c, f):
    pk = np.zeros((P, PACKW), np.float32)
    pk[:, COL_BQ1:COL_BQ1 + NK] = f("bq1").reshape(NK, P).T
    pk[:, COL_BQ2:COL_BQ2 + NK] = f("bq2").reshape(NK, P).T
    pk[:, COL_ONESCOL] = 1.0
    pk[:, COL_NEGHALF] = -0.5
    pk[:, COL_CID] = np.array([c], np.int32).view(np.float32)[0]
    pk[:, COL_IDENT:COL_IDENT + P] = np.eye(P, dtype=np.float32)
    pk[0, COL_BV2:COL_BV2 + D] = f("bv2")
    pk[0, COL_ONESROW:COL_ONESROW + P] = 1.0
    return pk


def kernel(**inputs):
    from concourse.bass_utils import run_bass_kernel_spmd

    f = lambda k: np.ascontiguousarray(np.asarray(inputs[k], dtype=np.float32))
    x0 = f("x")[0]                       # [S, D]; batches 1..7 are dead
    xT = np.ascontiguousarray(x0.T)      # [D, S]
    base = {
        "x0": x0,
        "Wk1": f("Wk1"), "Wq1": f("Wq1"), "Wk2": f("Wk2"), "Wq2": f("Wq2"),
        "Wv2": f("Wv2"),
    }
    in_maps = [
        {**base,
         "xTq": np.ascontiguousarray(xT[:, c * SH:(c + 1) * SH]),
         "pack": _pack(c, f)}
        for c in range(C)
    ]

    if "nc" not in _cache:
        _cache["nc"] = _build()
    res = run_bass_kernel_spmd(_cache["nc"], in_maps, list(range(C)))
    return res.results[0]["out"].astype(np.float32)


if __name__ == "__main__":
    d = np.load("/root/problem/inputs.npz")
    out = kernel(**{k: d[k] for k in d.files})
    ref = np.load("/root/problem/ref_out.npy")
    rel = np.abs(out - ref).max() / np.abs(ref).max()
    print("Relative error:", rel)


# revision 23
# speedup vs baseline: 1.0492x; 1.0101x over previous
"""Trainium2 Bass kernel for nn_ModelAttention2Layers (B=8, S=2048, D=512, K=256).

Only batch 0 matters (the reference returns final[0, -1, :]), so the 2048-query
sequence of batch 0 is sharded across the 8 cores (256 queries each).

All cross-core data movement uses relative-addressed remote_dma_broadcast
(SBUF -> SBUF) instead of collective_compute (15us fixed overhead + 40GB/s
each in the perf model). Each allgather is a tc.Switch on the core id: core j
issues one 8-destination broadcast (self included) whose out slot is j, so
slot j always holds core j's shard (keys stay in linear order):
  - block 1's k1 shards (k1 = Wk1^T x is sharded, not recomputed 8x; this
    also removes the 4MB full-xT load),
  - block 2's k2 and v2 shards,
  - hidden[-1] (core 7's last row; 16B/partition),
  - block 3's flash-style [o|l] partials, summed on every core.
Receive-side ordering: a Pool nop "gate" gets a wait on the remote semaphore
injected AFTER tile scheduling (the scheduling sim cannot satisfy
remotely-incremented semaphores), and every reader of a gathered tile gets an
explicit dependency edge on the gate.

Activation-table discipline: only {Exp, Ln, Square, Copy} are used (one
act-func table -> no 1.3us table reloads); 1/sqrt(s) = exp(-0.5*ln(s)).
Matmuls run in float32r / bf16 (full PE rate); k-projection biases are dropped
(softmax-invariant; they are zero in setup_inputs anyway).
"""
import sys

sys.path.insert(0, "/opt/trn_rl_repo")

import numpy as np

S, D, K, P, C = 2048, 512, 256, 128, 8
SH = S // C          # 256 queries/keys per core
ND, NK, NS, NSH = D // P, K // P, S // P, SH // P   # 4, 2, 16, 2
TRN2_NC_BASE = (0, 1, 2, 3, 6, 7, 4, 5)
RDESTS = [(0, TRN2_NC_BASE[s]) for s in range(C)]   # relative, self included
RSEM_TARGET = C * (16 // C)                          # 8 senders x 2

# packed-constants layout (one [P, PACKW] f32 DMA): see _pack() below
COL_BQ1, COL_BQ2 = 0, NK
COL_ONESCOL = 2 * NK
COL_NEGHALF = 2 * NK + 1
COL_CID = 2 * NK + 2
COL_IDENT = 2 * NK + 3
COL_BV2 = COL_IDENT + P          # row 0 only
COL_ONESROW = COL_BV2 + D        # row 0 only
PACKW = COL_ONESROW + P

_cache = {}


def _steer_act_tables():
    """Make the act-table insertion pass resolve Ln to
    natural_log_exp_and_others (which also holds Exp/Square/Copy) instead of
    first-match natural_log: hide Ln from every other table. Table order and
    indices are unchanged, so the emitted act_func_set_id still addresses the
    real act_info.json entry (which genuinely contains Ln). Returns a restore
    thunk."""
    import concourse.bacc as bacc_mod
    from concourse import mybir

    orig = bacc_mod.get_activation_tables

    def steered(arch):
        tabs = dict(orig(arch))
        ln = mybir.ActivationFunctionType.Ln
        exp = mybir.ActivationFunctionType.Exp
        for name, funcs in tabs.items():
            if ln in funcs and exp not in funcs:
                tabs[name] = funcs - {ln}
            elif exp in funcs and ln not in funcs:
                tabs[name] = funcs - {exp}
        return tabs

    bacc_mod.get_activation_tables = steered
    return lambda: setattr(bacc_mod, "get_activation_tables", orig)


def _build():
    import concourse.bass as bass
    import concourse.tile as tile
    from concourse import mybir, bacc
    from bass_rust import add_dep_helper, RuntimeValue

    F32 = mybir.dt.float32
    F32R = mybir.dt.float32r
    BF16 = mybir.dt.bfloat16
    I32 = mybir.dt.int32
    AF = mybir.ActivationFunctionType
    ts = bass.ts

    nc = bacc.Bacc()

    ins = {}
    for name, shape, dt in [
        ("x0", [S, D], F32), ("xTq", [D, SH], F32),
        ("Wk1", [D, K], F32), ("Wq1", [D, K], F32), ("Wk2", [D, K], F32),
        ("Wq2", [D, K], F32), ("Wv2", [D, D], F32),
        ("pack", [P, PACKW], F32),
    ]:
        ins[name] = nc.dram_tensor(name, shape, dt, kind="ExternalInput")
    out_ext = nc.dram_tensor("out", [D], F32, kind="ExternalOutput")

    gates = []  # (nop instruction, semaphore, target) -> wait injected post-schedule

    with tile.TileContext(nc) as tc:
        with tc.tile_pool(name="const", bufs=1) as cw, \
             tc.tile_pool(name="big", bufs=1) as big, \
             tc.tile_pool(name="work", bufs=1) as wk, \
             tc.tile_pool(name="send", bufs=1) as snd, \
             tc.tile_pool(name="gath", bufs=1) as gth, \
             tc.tile_pool(name="pp", bufs=2) as pp, \
             tc.tile_pool(name="small", bufs=2) as sm, \
             tc.tile_pool(name="scp", bufs=2, space="PSUM") as scp, \
             tc.tile_pool(name="mmp", bufs=2, space="PSUM") as mmp, \
             tc.tile_pool(name="tpp", bufs=2, space="PSUM") as tpp:

            rsem_k1 = nc.alloc_semaphore("rsem_k1")
            rsem_k2 = nc.alloc_semaphore("rsem_k2")
            rsem_v2 = nc.alloc_semaphore("rsem_v2")
            rsem_hl = nc.alloc_semaphore("rsem_hl")
            rsem_ol = nc.alloc_semaphore("rsem_ol")
            lsem = nc.alloc_semaphore("lsem")

            # ---- input loads ----
            # f32r tiles are loaded with a bitcast (same bits); SP carries
            # xTq + x0, Act carries the weights + the packed constants.
            # Pool stays free for remote-DMA desc generation.
            xTq_r = cw.tile([P, ND, SH], F32R)
            nc.sync.dma_start(xTq_r[:],
                              ins["xTq"][:].bitcast(F32R).rearrange("(k p) j -> p k j", p=P))
            x0_r = big.tile([P, NS, D], F32R, tag="XV")
            x0_dmas = []
            for cb in range(4):
                x0_dmas.append(nc.sync.dma_start(
                    x0_r[:, 4 * cb:4 * cb + 4, :],
                    ins["x0"][:].bitcast(F32R).rearrange("(n p) d -> p n d", p=P)[:, 4 * cb:4 * cb + 4, :]))
            pk = cw.tile([P, PACKW], F32)
            nc.scalar.dma_start(pk[:], ins["pack"][:])
            W_r = {}
            w_dmas = {}
            for w, ncol in [("Wk1", K), ("Wq1", K), ("Wk2", K), ("Wq2", K),
                            ("Wv2", D)]:
                W_r[w] = cw.tile([P, ND, ncol], F32R, name=f"W_{w}", tag=f"W_{w}")
                w_dmas[w] = nc.scalar.dma_start(
                    W_r[w][:], ins[w][:].bitcast(F32R).rearrange("(k p) n -> p k n", p=P))
            for w in ("Wk2", "Wq2", "Wv2"):
                add_dep_helper(w_dmas[w].ins, x0_dmas[-1].ins, sync=True,
                               reason="x0 before late weights")
            bq1_sb = pk[:, COL_BQ1:COL_BQ1 + NK]
            bq2_sb = pk[:, COL_BQ2:COL_BQ2 + NK]
            onescol_f = pk[:, COL_ONESCOL:COL_ONESCOL + 1]
            neghalf = pk[:, COL_NEGHALF:COL_NEGHALF + 1]
            ident_r = pk[:, COL_IDENT:COL_IDENT + P].bitcast(F32R)
            bv2_r = pk[0:1, COL_BV2:COL_BV2 + D].bitcast(F32R)
            ones_r = pk[0:1, COL_ONESROW:COL_ONESROW + P].bitcast(F32R)

            cid_reg = nc.gpsimd.alloc_register("cid")
            nc.gpsimd.reg_load(cid_reg, pk[0:1, COL_CID:COL_CID + 1].bitcast(I32))
            cid_val = RuntimeValue(cid_reg, min_val=0, max_val=C - 1)

            triggers = []

            def bcast_send(full_tile, src_tile, rsem, name):
                """Switch on core id; core j broadcasts src into slot j of
                full_tile on all 8 cores (self included)."""
                for j in tc.Switch(cid_val, C, hint=f"ag_{name}"):
                    nc.gpsimd.remote_dma_broadcast(
                        full_tile[:, j], src_tile[:],
                        remote_sem=rsem, local_sem=lsem, rdests=RDESTS)
                    triggers.append(nc.gpsimd.trigger_dma(count=None))

            def make_gate(rsem, name):
                """Pool nop that (post-scheduling) waits for all 8 broadcasts.
                Ordered after every trigger emitted so far, so a blocked gate
                never delays a send."""
                gate = nc.gpsimd.nop(nofuse=True, hint=f"gate_{name}")
                for t in triggers:
                    add_dep_helper(gate.ins, t.ins, sync=False,
                                   reason="sends before gate")
                gates.append((gate, rsem, RSEM_TARGET))
                return gate

            # ---- block 1 projections (sharded k1) ----
            k1s = snd.tile([P, NK, SH], F32R, tag="snd_k1")
            for m in range(NK):
                pm = mmp.tile([P, SH], F32, tag="mm")
                for k in range(ND):
                    nc.tensor.matmul(pm[:], W_r["Wk1"][:, k, ts(m, P)], xTq_r[:, k, :],
                                     start=(k == 0), stop=(k == ND - 1))
                nc.scalar.activation(k1s[:, m, :], pm[:], AF.Copy)
            k1_full = gth.tile([P, C, NK, SH], F32R, tag="g_k1", name="dbg_k1full")
            bcast_send(k1_full, k1s, rsem_k1, "k1")
            gate_k1 = make_gate(rsem_k1, "k1")

            q1T = wk.tile([P, NK, SH], F32R, tag="qT1", name="dbg_q1T")
            for m in range(NK):
                pm = mmp.tile([P, SH], F32, tag="mm")
                for k in range(ND):
                    nc.tensor.matmul(pm[:], W_r["Wq1"][:, k, ts(m, P)], xTq_r[:, k, :],
                                     start=(k == 0), stop=(k == ND - 1))
                nc.vector.tensor_scalar_add(q1T[:, m, :], pm[:], bq1_sb[:, m:m + 1])

            def attention(qT, kfull, v_blocks, out_dst, pt_dtype,
                          score_gate=None, av_gate=None, pool_reduce=False):
                """out_dst[:, qm, :] = softmax(q.k^T) @ V for this core's
                queries (keys in linear order). The two query blocks are
                software-pipelined through two [P, 1024] psum score slots."""
                def scores(qm, h):
                    sch = scp.tile([P, 4, SH], F32, tag="sc")
                    for jj in range(4):
                        j = 4 * h + jj
                        for dm in range(NK):
                            mm = nc.tensor.matmul(
                                sch[:, jj, :], qT[:, dm, ts(qm, P)],
                                kfull[:, j, dm, :],
                                start=(dm == 0), stop=(dm == NK - 1))
                            if score_gate is not None:
                                add_dep_helper(mm.ins, score_gate.ins,
                                               sync=True, reason="gathered keys")
                    return sch

                def creduce(sch, mx4, h):
                    flat = sch[:].rearrange("p a b -> p (a b)")
                    nc.vector.reduce_max(mx4[:, 2 * h:2 * h + 1], flat[:, 0:512],
                                         axis=mybir.AxisListType.X)
                    nc.vector.reduce_max(mx4[:, 2 * h + 1:2 * h + 2],
                                         flat[:, 512:1024],
                                         axis=mybir.AxisListType.X)

                def neg_max(mx4):
                    mx = sm.tile([P, 1], F32, tag="mx")
                    nc.vector.reduce_max(mx[:], mx4[:], axis=mybir.AxisListType.X)
                    nm = sm.tile([P, 1], F32, tag="nm")
                    nc.vector.tensor_scalar_mul(nm[:], mx[:], -1.0)
                    return nm

                def exp_half(sch, Pt, lsum, nm, h):
                    flat = sch[:].rearrange("p a b -> p (a b)")
                    nc.scalar.activation(Pt[:, h * 1024:(h + 1) * 1024], flat[:],
                                         AF.Exp, bias=nm[:],
                                         accum_out=lsum[:, h:h + 1])

                def transp(Pt, PT, glo, ghi):
                    for g in range(glo, ghi):
                        tp = tpp.tile([P, 4, P], F32R, tag="tp")
                        for u in range(4):
                            nc.tensor.transpose(tp[:, u, :], Pt[:, ts(4 * g + u, P)],
                                                ident_r)
                        nc.vector.tensor_copy(
                            PT[:, 4 * g:4 * g + 4, :].rearrange("p a b -> p (a b)"),
                            tp[:].rearrange("p a b -> p (a b)"))

                def av_out(PT, lsum, qm):
                    l = sm.tile([P, 1], F32, tag="l")
                    nc.vector.reduce_sum(l[:], lsum[:], axis=mybir.AxisListType.X)
                    rl = sm.tile([P, 1], F32, tag="rl")
                    nc.vector.reciprocal(rl[:], l[:])
                    av = mmp.tile([P, D], F32, tag="mm")
                    for n in range(NS):
                        mm = nc.tensor.matmul(av[:], PT[:, n, :], v_blocks[n],
                                              start=(n == 0), stop=(n == NS - 1))
                        if av_gate is not None:
                            add_dep_helper(mm.ins, av_gate.ins, sync=True,
                                           reason="gathered values")
                    nc.scalar.activation(out_dst[:, qm, :], av[:], AF.Copy,
                                         scale=rl[:])

                mx4_0 = sm.tile([P, 4], F32, tag="mx4")
                mx4_1 = sm.tile([P, 4], F32, tag="mx4")
                s0a = scores(0, 0); creduce(s0a, mx4_0, 0)
                s0b = scores(0, 1); creduce(s0b, mx4_0, 1)
                nm0 = neg_max(mx4_0)
                Pt0 = pp.tile([P, S], F32R, tag="P")
                lsum0 = sm.tile([P, 2], F32, tag="lsum")
                exp_half(s0a, Pt0, lsum0, nm0, 0)
                s1a = scores(1, 0); creduce(s1a, mx4_1, 0)
                exp_half(s0b, Pt0, lsum0, nm0, 1)
                PT0 = pp.tile([P, NS, P], pt_dtype, tag="PT")
                transp(Pt0, PT0, 0, 2)
                s1b = scores(1, 1); creduce(s1b, mx4_1, 1)
                transp(Pt0, PT0, 2, 4)
                av_out(PT0, lsum0, 0)
                nm1 = neg_max(mx4_1)
                Pt1 = pp.tile([P, S], F32R, tag="P")
                lsum1 = sm.tile([P, 2], F32, tag="lsum")
                exp_half(s1a, Pt1, lsum1, nm1, 0)
                exp_half(s1b, Pt1, lsum1, nm1, 1)
                PT1 = pp.tile([P, NS, P], pt_dtype, tag="PT")
                transp(Pt1, PT1, 0, 4)
                av_out(PT1, lsum1, 1)

            out1 = wk.tile([P, NSH, D], F32R, tag="H", name="dbg_out1")
            attention(q1T, k1_full, [x0_r[:, n, :] for n in range(NS)], out1, F32R,
                      score_gate=gate_k1, pool_reduce=True)

            def transpose_rows(src, hl_out=None):
                """src [P, NSH, D] -> dst [P, ND, SH]; optionally extract the
                last row (query SH-1) into hl_out [P, ND]."""
                dst = wk.tile([P, ND, SH], F32R, tag="HT")
                for qm in range(NSH):
                    tp = tpp.tile([P, 4, P], F32R, tag="tp")
                    for dm in range(ND):
                        nc.tensor.transpose(tp[:, dm, :], src[:, qm, ts(dm, P)],
                                            ident_r)
                    if qm == 0:
                        nc.scalar.activation(dst[:, :, ts(qm, P)], tp[:], AF.Copy)
                    else:
                        nc.vector.tensor_copy(dst[:, :, ts(qm, P)], tp[:])
                    if hl_out is not None and qm == NSH - 1:
                        nc.vector.tensor_copy(
                            hl_out[:],
                            tp[:, :, P - 1:P].rearrange("p a b -> p (a b)"))
                return dst

            out1T = transpose_rows(out1)

            # ---- block 2 shard projections + gathers ----
            k2T = snd.tile([P, NK, SH], BF16, tag="snd_k2")
            for m in range(NK):
                pm = mmp.tile([P, SH], F32, tag="mm")
                for k in range(ND):
                    nc.tensor.matmul(pm[:], W_r["Wk2"][:, k, ts(m, P)], out1T[:, k, :],
                                     start=(k == 0), stop=(k == ND - 1))
                nc.scalar.activation(k2T[:, m, :], pm[:], AF.Copy)
            k2_full = gth.tile([P, C, NK, SH], BF16, tag="g_k2", name="dbg_k2full")
            bcast_send(k2_full, k2T, rsem_k2, "k2")
            gate_k2 = make_gate(rsem_k2, "k2")

            q2T = wk.tile([P, NK, SH], BF16, tag="qT2")
            for m in range(NK):
                pm = mmp.tile([P, SH], F32, tag="mm")
                for k in range(ND):
                    nc.tensor.matmul(pm[:], W_r["Wq2"][:, k, ts(m, P)], out1T[:, k, :],
                                     start=(k == 0), stop=(k == ND - 1))
                nc.vector.tensor_scalar_add(q2T[:, m, :], pm[:], bq2_sb[:, m:m + 1])

            def rsqrt_act(dstap, srcap):
                """1/sqrt(s) via exp(-0.5*ln(s)) — stays in one act table."""
                t = sm.tile([P, 1], F32, tag="lnt")
                nc.scalar.activation(t[:], srcap, AF.Ln)
                nc.scalar.activation(dstap, t[:], AF.Exp, scale=neghalf)

            def vproj(hT, out_dtype, pool, tag, normalize):
                """v = h @ Wv2 + bv2 for this core's 256 rows; if normalize,
                rows are L2-normalized, else the raw rows and the 1/|row|
                factors (rn [P, NSH]) are returned separately."""
                v_sb = pool.tile([P, NSH, D], out_dtype, tag=tag)
                rn_t = None if normalize else sm.tile([P, NSH], F32, tag="rn3")
                for r in range(NSH):
                    pm = mmp.tile([P, D], F32, tag="mm")
                    for k in range(ND):
                        nc.tensor.matmul(pm[:], hT[:, k, ts(r, P)], W_r["Wv2"][:, k, :],
                                         start=(k == 0), stop=False)
                    nc.tensor.matmul(pm[:], ones_r, bv2_r, start=False, stop=True)
                    scr = sm.tile([P, D], F32, tag="scr")
                    ssum = sm.tile([P, 1], F32, tag="ssum")
                    nc.scalar.activation(scr[:], pm[:], AF.Square, accum_out=ssum[:])
                    if normalize:
                        rn = sm.tile([P, 1], F32, tag="rn")
                        rsqrt_act(rn[:], ssum[:])
                        nc.vector.tensor_scalar_mul(v_sb[:, r, :], pm[:], rn[:])
                    else:
                        rsqrt_act(rn_t[:, r:r + 1], ssum[:])
                        nc.vector.tensor_copy(v_sb[:, r, :], pm[:])
                return v_sb, rn_t

            v2, _ = vproj(out1T, BF16, snd, "snd_v2", normalize=True)
            v2_full = gth.tile([P, C, NSH, D], BF16, tag="g_v2", name="dbg_v2full")
            bcast_send(v2_full, v2, rsem_v2, "v2")
            gate_v2 = make_gate(rsem_v2, "v2")

            # ---- block 2 attention ----
            v2_blocks = [v2_full[:, n // NSH, n % NSH, :] for n in range(NS)]
            hidden = wk.tile([P, NSH, D], F32R, tag="H", name="dbg_hidden")
            attention(q2T, k2_full, v2_blocks, hidden, BF16,
                      score_gate=gate_k2, av_gate=gate_v2)

            hl_c = snd.tile([P, ND], F32R, tag="snd_hl")
            hT = transpose_rows(hidden, hl_out=hl_c)

            hlg = gth.tile([P, C, ND], F32R, tag="g_hl", name="dbg_hlg")
            bcast_send(hlg, hl_c, rsem_hl, "hl")
            gate_hl = make_gate(rsem_hl, "hl")

            # ---- block 3 (flash-style partials over this core's 256 keys).
            # k3/v3/rn3 only need local data and overlap the hl exchange; the
            # 1/|v| factors are folded into p3 so nothing heavy sits on the
            # post-hl critical path.
            k3T = wk.tile([P, NK, SH], F32R, tag="k3")
            for m in range(NK):
                pm = mmp.tile([P, SH], F32, tag="mm")
                for k in range(ND):
                    nc.tensor.matmul(pm[:], W_r["Wk2"][:, k, ts(m, P)], hT[:, k, :],
                                     start=(k == 0), stop=(k == ND - 1))
                nc.vector.tensor_copy(k3T[:, m, :], pm[:])
            v3, rn3 = vproj(hT, F32R, wk, "v3", normalize=False)

            # q3 = Wq2^T @ hidden[-1] + bq2 ; hidden[-1] is core 7's slot
            q3 = sm.tile([P, NK], F32R, tag="q3")
            for fm in range(NK):
                pm = mmp.tile([P, 1], F32, tag="mm")
                for dm in range(ND):
                    mm = nc.tensor.matmul(pm[:], W_r["Wq2"][:, dm, ts(fm, P)],
                                          hlg[:, C - 1, dm:dm + 1],
                                          start=(dm == 0), stop=(dm == ND - 1))
                    add_dep_helper(mm.ins, gate_hl.ins, sync=True, reason="hl gather")
                nc.vector.tensor_scalar_add(q3[:, fm:fm + 1], pm[:], bq2_sb[:, fm:fm + 1])

            # s3 (scores for my 256 keys; |s3| small so exp needs no max shift)
            s3p = tpp.tile([P, NSH], F32, tag="tp")
            for n in range(NSH):
                for fm in range(NK):
                    nc.tensor.matmul(s3p[:, n:n + 1], k3T[:, fm, ts(n, P)],
                                     q3[:, fm:fm + 1],
                                     start=(fm == 0), stop=(fm == NK - 1))
            p3e = sm.tile([P, NSH], F32, tag="p3e")
            nc.scalar.activation(p3e[:], s3p[:], AF.Exp)
            p3 = sm.tile([P, NSH], F32R, tag="p3")
            nc.vector.tensor_tensor(p3[:], p3e[:], rn3[:], mybir.AluOpType.mult)

            # partial numerator oT [128,4] (d on partitions) + replicated l
            ol_ps = mmp.tile([P, ND + 1], F32, tag="mm")
            for dm in range(ND):
                for n in range(NSH):
                    nc.tensor.matmul(ol_ps[:, dm:dm + 1], v3[:, n, ts(dm, P)],
                                     p3[:, n:n + 1],
                                     start=(n == 0), stop=(n == NSH - 1))
            l3p = tpp.tile([1, 1], F32, tag="tp")
            for n in range(NSH):
                nc.tensor.matmul(l3p[:], p3e[:, n:n + 1], onescol_f,
                                 start=(n == 0), stop=(n == NSH - 1))
            l3f = sm.tile([1, 1], F32R, tag="l3f")
            nc.vector.tensor_copy(l3f[:], l3p[:])
            nc.tensor.matmul(ol_ps[:, ND:ND + 1], ones_r, l3f[:],
                             start=True, stop=True)
            ol = snd.tile([P, ND + 1], F32, tag="snd_ol")
            nc.vector.tensor_copy(ol[:], ol_ps[:])

            olg = gth.tile([P, C, ND + 1], F32, tag="g_ol", name="dbg_olg")
            bcast_send(olg, ol, rsem_ol, "ol")
            gate_ol = make_gate(rsem_ol, "ol")

            tot = wk.tile([P, ND + 1], F32, tag="tot")
            rs = nc.vector.reduce_sum(tot[:], olg[:].rearrange("p c e -> p e c"),
                                      axis=mybir.AxisListType.X)
            add_dep_helper(rs.ins, gate_ol.ins, sync=True, reason="ol gather")
            rl3 = sm.tile([P, 1], F32, tag="rl3")
            nc.vector.reciprocal(rl3[:], tot[:, ND:ND + 1])
            fin = wk.tile([P, ND], F32, tag="fin")
            nc.vector.tensor_scalar_mul(fin[:], tot[:, 0:ND], rl3[:])
            nc.sync.dma_start(out_ext[:].rearrange("(k p) -> p k", p=P), fin[:])

    for gate, sem, target in gates:
        gate.wait_op(sem, target, "sem-ge")
    restore = _steer_act_tables()
    try:
        nc.finalize()
    finally:
        restore()
    return nc


def _pack(c, f):
    pk = np.zeros((P, PACKW), np.float32)
    pk[:, COL_BQ1:COL_BQ1 + NK] = f("bq1").reshape(NK, P).T
    pk[:, COL_BQ2:COL_BQ2 + NK] = f("bq2").reshape(NK, P).T
    pk[:, COL_ONESCOL] = 1.0
    pk[:, COL_NEGHALF] = -0.5
    pk[:, COL_CID] = np.array([c], np.int32).view(np.float32)[0]
    pk[:, COL_IDENT:COL_IDENT + P] = np.eye(P, dtype=np.float32)
    pk[0, COL_BV2:COL_BV2 + D] = f("bv2")
    pk[0, COL_ONESROW:COL_ONESROW + P] = 1.0
    return pk


def kernel(**inputs):
    from concourse.bass_utils import run_bass_kernel_spmd

    f = lambda k: np.ascontiguousarray(np.asarray(inputs[k], dtype=np.float32))
    x0 = f("x")[0]                       # [S, D]; batches 1..7 are dead
    xT = np.ascontiguousarray(x0.T)      # [D, S]
    base = {
        "x0": x0,
        "Wk1": f("Wk1"), "Wq1": f("Wq1"), "Wk2": f("Wk2"), "Wq2": f("Wq2"),
        "Wv2": f("Wv2"),
    }
    in_maps = [
        {**base,
         "xTq": np.ascontiguousarray(xT[:, c * SH:(c + 1) * SH]),
         "pack": _pack(c, f)}
        for c in range(C)
    ]

    if "nc" not in _cache:
        _cache["nc"] = _build()
    res = run_bass_kernel_spmd(_cache["nc"], in_maps, list(range(C)))
    return res.results[0]["out"].astype(np.float32)


if __name__ == "__main__":
    d = np.load("/root/problem/inputs.npz")
    out = kernel(**{k: d[k] for k in d.files})
    ref = np.load("/root/problem/ref_out.npy")
    rel = np.abs(out - ref).max() / np.abs(ref).max()
    print("Relative error:", rel)
